# revision 17
# baseline (speedup 1.0000x reference)
"""Trainium2 Bass kernel for nn_AttentionDecoder (Bahdanau attention + GRU greedy decoder).

Sharding: pure data parallel, B=2048 split as 256 rows per core across 8 cores.
All compute in bf16 with f32 PSUM accumulation (verified: rel err ~6e-3, no argmax flips).

Layout scheme (per core, BL=256):
  - partitions packed as p = 32*(b%4) + t  ("bd layout") so block-diagonal attention
    matmuls pack 4 batch rows per matmul; free index g = b//4 makes everything b-major.
  - attention + fc1 feature-major (feature on partitions), GRU gates batch-major,
    bridged by PE transposes.
  - context and energy matmuls run "flipped": enc/att chunks are the stationary
    operand (M=128, dense base-0 PSUM output), attn_bd/v the moving operand.
  - sigmoid computed as (1+tanh(x/2))/2 so tanh/exp/relu/copy share one ACT
    table set (no per-step ACT_TABLE_LOAD).
"""

import os
import threading
import numpy as np
import ml_dtypes

N_CORES = 8
B, T, ENC = 2048, 32, 512
DEC, ATT, EMB, NCLS, L = 256, 256, 64, 37, 10
BL = B // N_CORES  # 256 per core

_BF = ml_dtypes.bfloat16

_lock = threading.Lock()
_cache = {}


def _build():
    import concourse.bass as bass
    import concourse.tile as tile
    from concourse import bacc, mybir

    bf = mybir.dt.bfloat16
    f32 = mybir.dt.float32

    nc = bacc.Bacc("TRN2", target_bir_lowering=False, debug=False,
                   num_devices=N_CORES)

    # ---------------- DRAM parameters ----------------
    d_enc = nc.dram_tensor("enc", [BL, T, ENC], bf, kind="ExternalInput").ap()
    d_wdec = nc.dram_tensor("w_dec", [DEC, ATT], bf, kind="ExternalInput").ap()
    d_wenc = nc.dram_tensor("w_enc", [ENC, ATT], bf, kind="ExternalInput").ap()
    d_v = nc.dram_tensor("v", [ATT, 1], bf, kind="ExternalInput").ap()
    d_embWb = nc.dram_tensor("embWb", [NCLS + 1, 3 * DEC], bf, kind="ExternalInput").ap()
    d_wihc = nc.dram_tensor("w_ih_c", [ENC, 3 * DEC], bf, kind="ExternalInput").ap()
    d_whhrz = nc.dram_tensor("w_hh_rz", [DEC, 2 * DEC], bf, kind="ExternalInput").ap()
    d_whhn = nc.dram_tensor("w_hh_n", [DEC, DEC], bf, kind="ExternalInput").ap()
    d_bhhn = nc.dram_tensor("b_hh_n", [1, DEC], bf, kind="ExternalInput").ap()
    d_fc1h = nc.dram_tensor("fc1_w_h", [DEC, DEC], bf, kind="ExternalInput").ap()
    d_fc1c = nc.dram_tensor("fc1_w_c", [ENC, DEC], bf, kind="ExternalInput").ap()
    d_fc1b = nc.dram_tensor("fc1_b", [DEC, 1], f32, kind="ExternalInput").ap()
    d_fc2w = nc.dram_tensor("fc2_w", [DEC, NCLS], bf, kind="ExternalInput").ap()
    d_fc2b = nc.dram_tensor("fc2_b", [1, NCLS], bf, kind="ExternalInput").ap()
    d_ihw = nc.dram_tensor("init_h_w", [ENC, DEC], bf, kind="ExternalInput").ap()
    d_ihb = nc.dram_tensor("init_h_b", [DEC, 1], f32, kind="ExternalInput").ap()
    d_out = nc.dram_tensor("out", [BL, L, NCLS], f32, kind="ExternalOutput").ap()

    # constants baked into the NEFF
    ident_np = np.eye(128, dtype=_BF)
    d_ident = nc.inline_tensor(ident_np, name="ident").ap()
    onesbd_np = np.zeros((128, 4), dtype=_BF)
    for bs in range(4):
        onesbd_np[32 * bs:32 * bs + 32, bs] = 1.0 / 32.0
    d_onesbd = nc.inline_tensor(onesbd_np, name="onesbd").ap()
    d_onesrow = nc.inline_tensor(np.ones((1, 256), dtype=_BF), name="onesrow").ap()

    AluOp = mybir.AluOpType
    ActF = mybir.ActivationFunctionType

    with tile.TileContext(nc) as tc:
        with (
            tc.tile_pool(name="persist", bufs=1) as P,
            tc.tile_pool(name="wpool", bufs=1) as W,
            tc.tile_pool(name="trans", bufs=3) as TR,
            tc.tile_pool(name="small", bufs=3) as SM,
            tc.tile_pool(name="ps", bufs=3, space="PSUM") as PS,
        ):
            # ---------------- persistent SBUF tensors ----------------
            enc_bd = P.tile([128, 64, ENC], bf, tag="enc_bd")       # 64KB/part
            ep = P.tile([128, 2, T, 256], bf, tag="ep")             # enc_proj^T, t-major
            att = P.tile([128, 2, T, 256], bf, tag="att")           # tanh buffer
            hT = P.tile([128, 2, BL], bf, tag="hT")                 # h feature-major
            h_b = P.tile([128, 2, DEC], bf, tag="h_b")              # h batch-major
            ctxT = P.tile([128, 4, BL], bf, tag="ctxT")             # context feature-major
            onehotT = P.tile([NCLS + 1, BL], bf, tag="onehotT")
            out_sb = P.tile([128, 2, L, NCLS], f32, tag="out_sb")

            # ---------------- weights to SBUF ----------------
            def wload(tag, shape, src, rearr=None):
                t = W.tile(shape, bf, tag=tag)
                nc.sync.dma_start(t[:], src if rearr is None else src.rearrange(rearr, p=128))
                return t

            w_dec = wload("w_dec", [128, 2, ATT], d_wdec, "(k p) n -> p k n")
            w_enc = wload("w_enc", [128, 4, ATT], d_wenc, "(k p) n -> p k n")
            v_sb = wload("v_sb", [128, 2, 1], d_v, "(k p) n -> p k n")
            embWb = wload("embWb", [NCLS + 1, 3 * DEC], d_embWb)
            w_ihc = wload("w_ihc", [128, 4, 3 * DEC], d_wihc, "(k p) n -> p k n")
            w_hhrz = wload("w_hhrz", [128, 2, 2 * DEC], d_whhrz, "(k p) n -> p k n")
            w_hhn = wload("w_hhn", [128, 2, DEC], d_whhn, "(k p) n -> p k n")
            bhhn = wload("bhhn", [1, DEC], d_bhhn)
            fc1h = wload("fc1h", [128, 2, DEC], d_fc1h, "(k p) n -> p k n")
            fc1c = wload("fc1c", [128, 4, DEC], d_fc1c, "(k p) n -> p k n")
            fc2w = wload("fc2w", [128, 2, NCLS], d_fc2w, "(k p) n -> p k n")
            fc2b = wload("fc2b", [1, NCLS], d_fc2b)
            ihw = wload("ihw", [128, 4, DEC], d_ihw, "(k p) n -> p k n")
            ident = wload("ident", [128, 128], d_ident)
            onesbd = wload("onesbd", [128, 4], d_onesbd)
            ihb = W.tile([128, 2, 1], f32)
            nc.sync.dma_start(ihb[:], d_ihb.rearrange("(k p) n -> p k n", p=128))
            fc1b = W.tile([128, 2, 1], f32)
            nc.sync.dma_start(fc1b[:], d_fc1b.rearrange("(k p) n -> p k n", p=128))
            ones1 = W.tile([1, 128], bf)
            nc.sync.dma_start(ones1[:], d_onesrow[:, 0:128])

            # ---------------- enc DMA into bd layout ----------------
            # enc_bd[32*bs+t, g, e] = enc[4g+bs, t, e]
            for bs in range(4):
                nc.sync.dma_start(
                    enc_bd[32 * bs:32 * bs + 32, :, :],
                    d_enc[bs::4].rearrange("g t e -> t g e"),
                )

            # ---------------- prologue: transposes + enc_proj ----------------
            for c in range(16):
                encTc = TR.tile([128, 4, 512], bf, tag="encTc")
                for eb in range(4):
                    tp = PS.tile([128, 512], bf, tag="a")
                    for gr in range(4):
                        g = 4 * c + gr
                        nc.tensor.transpose(
                            tp[:, 128 * gr:128 * gr + 128],
                            enc_bd[:, g, 128 * eb:128 * eb + 128],
                            ident[:],
                        )
                    if eb % 2 == 0:
                        nc.vector.tensor_copy(encTc[:, eb, :], tp[:])
                    else:
                        nc.scalar.copy(encTc[:, eb, :], tp[:])
                for ab in range(2):
                    pp = PS.tile([128, 512], f32, tag="a")
                    for eb in range(4):
                        nc.tensor.matmul(
                            pp[:],
                            w_enc[:, eb, 128 * ab:128 * ab + 128],
                            encTc[:, eb, :],
                            start=(eb == 0), stop=(eb == 3),
                        )
                    # evac to ep (t-major): chunk covers b in [16c, 16c+16)
                    dst = ep[:, ab, :, 16 * c:16 * c + 16].rearrange("p t b -> p b t")
                    if ab == 0:
                        nc.vector.tensor_copy(dst, pp[:].rearrange("p (b t) -> p b t", t=32))
                    else:
                        nc.scalar.copy(dst, pp[:].rearrange("p (b t) -> p b t", t=32))

            # ---------------- h0 ----------------
            meanT = TR.tile([128, 4, BL], bf, tag="meanT")
            for eb in range(4):
                mp = PS.tile([128, BL], f32, tag="a")
                for g in range(64):
                    nc.tensor.matmul(mp[:, 4 * g:4 * g + 4],
                                     enc_bd[:, g, 128 * eb:128 * eb + 128],
                                     onesbd[:], start=True, stop=True)
                nc.vector.tensor_copy(meanT[:, eb, :], mp[:])
            for db in range(2):
                hp = PS.tile([128, BL], f32, tag="a")
                for eb in range(4):
                    nc.tensor.matmul(hp[:], ihw[:, eb, 128 * db:128 * db + 128],
                                     meanT[:, eb, :], start=(eb == 0), stop=(eb == 3))
                nc.scalar.activation(hT[:, db, :], hp[:], ActF.Tanh, bias=ihb[:, db, :])
            for half in range(2):
                for db in range(2):
                    tp = PS.tile([128, 128], bf, tag="b", bufs=3)
                    nc.tensor.transpose(tp[:], hT[:, db, 128 * half:128 * half + 128],
                                        ident[:])
                    nc.vector.tensor_copy(h_b[:, half, 128 * db:128 * db + 128], tp[:])

            # onehotT init: y0 = 0 -> row 0 ones; row 37 = bias row (always 1)
            nc.vector.memset(onehotT[0:NCLS, :], 0)
            nc.vector.memset(onehotT[0:1, :], 1.0)
            nc.sync.dma_start(onehotT[NCLS:NCLS + 1, :], d_onesrow[:])

            # ---------------- decode loop ----------------
            # emitted per b-half: the two halves form independent dependency
            # chains within a step, so Tile overlaps half-1 elementwise
            # (DVE/ACT) with half-0 matmuls (PE) and vice versa.
            for step in range(L):
                decT = SM.tile([128, 2, BL], bf, tag="decT")
                n_sb = SM.tile([128, 2, DEC], bf, tag="n_sb")
                tz_sb = SM.tile([128, 2, DEC], bf, tag="tz_sb")
                hidT = SM.tile([128, 2, BL], bf, tag="hidT")
                attnT = SM.tile([32, BL], bf, tag="attnT")
                attn_bd = SM.tile([128, 64, 4], bf, tag="attn_bd")
                nc.vector.memset(attn_bd[:], 0)
                for half in range(2):
                    hs, he = 128 * half, 128 * half + 128
                    # dec_proj^T for this half
                    for ab in range(2):
                        dp = PS.tile([128, 128], f32, tag="b")
                        for db in range(2):
                            nc.tensor.matmul(dp[:],
                                             w_dec[:, db, 128 * ab:128 * ab + 128],
                                             hT[:, db, hs:he],
                                             start=(db == 0), stop=(db == 1))
                        nc.scalar.copy(decT[:, ab, hs:he], dp[:])
                    # s = ep + dec (broadcast over t); tanh in place (one op/half)
                    bcast = decT[:, :, hs:he].rearrange(
                        "p a (o b) -> p a o b", o=1).broadcast_to([128, 2, T, 128])
                    nc.vector.tensor_tensor(att[:, :, :, hs:he], ep[:, :, :, hs:he],
                                            bcast, op=AluOp.add)
                    nc.scalar.activation(att[:, :, :, hs:he], att[:, :, :, hs:he],
                                         ActF.Tanh)
                    # energy (batch-major) via flipped vdot
                    ebp = PS.tile([128, T], f32, tag="b")
                    for t in range(T):
                        for ab in range(2):
                            nc.tensor.matmul(ebp[:, t:t + 1], att[:, ab, t, hs:he],
                                             v_sb[:, ab, :],
                                             start=(ab == 0), stop=(ab == 1))
                    expB = SM.tile([128, T], bf, tag="expB")
                    nc.scalar.activation(expB[:], ebp[:], ActF.Exp)
                    zc = SM.tile([128, 1], f32, tag="zc")
                    nc.vector.tensor_reduce(zc[:], expB[:], axis=mybir.AxisListType.X,
                                            op=AluOp.add)
                    rz = SM.tile([128, 1], f32, tag="rz")
                    nc.vector.reciprocal(rz[:], zc[:])
                    attnB = SM.tile([128, T], bf, tag="attnB")
                    nc.vector.tensor_scalar(attnB[:], expB[:], rz[:], None,
                                            op0=AluOp.mult)
                    tp = PS.tile([32, 128], bf, tag="b")
                    nc.tensor.transpose(tp[:], attnB[:], ident[:])
                    nc.vector.tensor_copy(attnT[:, hs:he], tp[:])
                    # attn blockdiag build for this half
                    for bs in range(4):
                        nc.vector.tensor_copy(
                            attn_bd[32 * bs:32 * bs + 32,
                                    32 * half:32 * half + 32, bs],
                            attnT[:, hs + bs:he:4],
                        )
                    # context feature-major (flipped blockdiag)
                    for eb in range(4):
                        cp = PS.tile([128, 128], f32, tag="cp", bufs=2)
                        for gr in range(32):
                            g = 32 * half + gr
                            nc.tensor.matmul(cp[:, 4 * gr:4 * gr + 4],
                                             enc_bd[:, g, 128 * eb:128 * eb + 128],
                                             attn_bd[:, g, :], start=True, stop=True)
                        nc.vector.tensor_copy(ctxT[:, eb, hs:he], cp[:])
                    # GRU gates
                    gi_rz = PS.tile([128, 2 * DEC], f32, tag="a")
                    gi_n = PS.tile([128, DEC], f32, tag="b")
                    ghn = PS.tile([128, DEC], f32, tag="b")
                    oh = onehotT[:, hs:he]
                    nc.tensor.matmul(gi_rz[:], oh, embWb[:, 0:512],
                                     start=True, stop=False)
                    nc.tensor.matmul(gi_n[:], oh, embWb[:, 512:768],
                                     start=True, stop=False)
                    for eb in range(4):
                        ct = ctxT[:, eb, hs:he]
                        nc.tensor.matmul(gi_rz[:], ct, w_ihc[:, eb, 0:512],
                                         start=False, stop=False)
                        nc.tensor.matmul(gi_n[:], ct, w_ihc[:, eb, 512:768],
                                         start=False, stop=False)
                    nc.tensor.matmul(ghn[:], ones1[:], bhhn[:], start=True, stop=False)
                    nc.tensor.matmul(gi_n[:], ones1[:], bhhn[:], start=False, stop=False)
                    for db in range(2):
                        hTs = hT[:, db, hs:he]
                        nc.tensor.matmul(gi_rz[:], hTs, w_hhrz[:, db, :], start=False,
                                         stop=(db == 1))
                        nc.tensor.matmul(ghn[:], hTs, w_hhn[:, db, :], start=False,
                                         stop=(db == 1))
                        nc.tensor.matmul(gi_n[:], hTs, w_hhn[:, db, :], start=False,
                                         stop=(db == 1))
                    # r-gate via tanh: npre = gi_n + tanh(rx/2)*ghn2
                    tr_sb = SM.tile([128, DEC], bf, tag="tr_sb")
                    nc.scalar.activation(tr_sb[:], gi_rz[:, 0:DEC], ActF.Tanh, scale=0.5)
                    nc.scalar.activation(tz_sb[:, half, :], gi_rz[:, DEC:2 * DEC],
                                         ActF.Tanh, scale=0.5)
                    rhn = SM.tile([128, DEC], bf, tag="rhn")
                    nc.vector.tensor_tensor(rhn[:], tr_sb[:], ghn[:], op=AluOp.mult)
                    npre = SM.tile([128, DEC], bf, tag="npre")
                    nc.vector.tensor_tensor(npre[:], gi_n[:], rhn[:], op=AluOp.add)
                    nc.scalar.activation(n_sb[:, half, :], npre[:], ActF.Tanh)
                    # h_new = 0.5*(t1 + tz*t1) + n,  t1 = h - n
                    t1 = SM.tile([128, DEC], bf, tag="t1")
                    nc.vector.tensor_tensor(t1[:], h_b[:, half, :], n_sb[:, half, :],
                                            op=AluOp.subtract)
                    t2 = SM.tile([128, DEC], bf, tag="t2")
                    nc.vector.tensor_tensor(t2[:], tz_sb[:, half, :], t1[:],
                                            op=AluOp.mult)
                    t3 = SM.tile([128, DEC], bf, tag="t3")
                    nc.vector.tensor_tensor(t3[:], t1[:], t2[:], op=AluOp.add)
                    nc.vector.scalar_tensor_tensor(h_b[:, half, :], t3[:], 0.5,
                                                   n_sb[:, half, :],
                                                   op0=AluOp.mult, op1=AluOp.add)
                    for db in range(2):
                        tp = PS.tile([128, 128], bf, tag="b")
                        nc.tensor.transpose(tp[:],
                                            h_b[:, half, 128 * db:128 * db + 128],
                                            ident[:])
                        nc.vector.tensor_copy(hT[:, db, hs:he], tp[:])
                    # fc1 feature-major (this half's columns)
                    for db in range(2):
                        fp = PS.tile([128, 128], f32, tag="b")
                        for k in range(2):
                            nc.tensor.matmul(fp[:], fc1h[:, k, 128 * db:128 * db + 128],
                                             hT[:, k, hs:he], start=(k == 0), stop=False)
                        for eb in range(4):
                            nc.tensor.matmul(fp[:], fc1c[:, eb, 128 * db:128 * db + 128],
                                             ctxT[:, eb, hs:he], start=False,
                                             stop=(eb == 3))
                        nc.scalar.activation(hidT[:, db, hs:he], fp[:], ActF.Relu,
                                             bias=fc1b[:, db, :])
                    # logits + argmax onehot
                    lp = PS.tile([128, NCLS], f32, tag="b")
                    nc.tensor.matmul(lp[:], ones1[:], fc2b[:], start=True, stop=False)
                    for db in range(2):
                        nc.tensor.matmul(lp[:], hidT[:, db, hs:he],
                                         fc2w[:, db, :], start=False, stop=(db == 1))
                    nc.scalar.copy(out_sb[:, half, step, :], lp[:])
                    if step < L - 1:
                        mx = SM.tile([128, 1], f32, tag="mx")
                        nc.vector.tensor_reduce(mx[:], lp[:], axis=mybir.AxisListType.X,
                                                op=AluOp.max)
                        ohB = SM.tile([128, NCLS], bf, tag="ohB")
                        nc.vector.tensor_tensor(
                            ohB[:], lp[:],
                            mx[:].broadcast_to([128, NCLS]), op=AluOp.is_equal)
                        tp = PS.tile([NCLS, 128], bf, tag="b")
                        nc.tensor.transpose(tp[:], ohB[:], ident[:])
                        nc.vector.tensor_copy(onehotT[0:NCLS, hs:he], tp[:])

            # ---------------- output DMA ----------------
            for half in range(2):
                nc.sync.dma_start(
                    d_out[128 * half:128 * half + 128],
                    out_sb[:, half, :, :],
                )

    nc.compile()
    return nc


def _get_nc():
    with _lock:
        if "nc" not in _cache:
            _cache["nc"] = _build()
        return _cache["nc"]


def kernel(**inputs):
    nc = _get_nc()
    from concourse.bass_utils import run_bass_kernel_spmd

    enc = np.ascontiguousarray(inputs["encoder_outputs"], dtype=np.float32)
    emb = inputs["emb"].astype(np.float32)
    W_enc = inputs["W_enc"].astype(np.float32)
    W_dec = inputs["W_dec"].astype(np.float32)
    v = inputs["v"].astype(np.float32)
    init_h_W = inputs["init_h_W"].astype(np.float32)
    init_h_b = inputs["init_h_b"].astype(np.float32)
    W_ih = inputs["W_ih"].astype(np.float32)
    b_ih = inputs["b_ih"].astype(np.float32)
    W_hh = inputs["W_hh"].astype(np.float32)
    b_hh = inputs["b_hh"].astype(np.float32)
    fc1_W = inputs["fc1_W"].astype(np.float32)
    fc1_b = inputs["fc1_b"].astype(np.float32)
    fc2_W = inputs["fc2_W"].astype(np.float32)
    fc2_b = inputs["fc2_b"].astype(np.float32)

    # host precompute: embedding projected through W_ih (emb part) + rz biases;
    # W_hh_n/b_hh_n halved for the tanh-form sigmoid r-gate
    bias_row = np.concatenate([(b_ih + b_hh)[:2 * DEC], b_ih[2 * DEC:]])
    embWb = np.concatenate([emb @ W_ih[:EMB], bias_row[None, :]], axis=0)

    bfc = lambda a: np.ascontiguousarray(a, dtype=_BF)
    shared = {
        "w_dec": bfc(W_dec),
        "w_enc": bfc(W_enc),
        "v": bfc(v.reshape(ATT, 1)),
        "embWb": bfc(embWb),
        "w_ih_c": bfc(W_ih[EMB:]),
        "w_hh_rz": bfc(W_hh[:, :2 * DEC]),
        "w_hh_n": bfc(0.5 * W_hh[:, 2 * DEC:]),
        "b_hh_n": bfc(0.5 * b_hh[2 * DEC:].reshape(1, DEC)),
        "fc1_w_h": bfc(fc1_W[:DEC]),
        "fc1_w_c": bfc(fc1_W[DEC:]),
        "fc1_b": np.ascontiguousarray(fc1_b.reshape(DEC, 1), dtype=np.float32),
        "fc2_w": bfc(fc2_W),
        "fc2_b": bfc(fc2_b.reshape(1, NCLS)),
        "init_h_w": bfc(init_h_W),
        "init_h_b": np.ascontiguousarray(init_h_b.reshape(DEC, 1), dtype=np.float32),
    }
    enc_bf = enc.astype(_BF)
    in_maps = []
    for i in range(N_CORES):
        m = dict(shared)
        m["enc"] = np.ascontiguousarray(enc_bf[i * BL:(i + 1) * BL])
        in_maps.append(m)

    res = run_bass_kernel_spmd(nc, in_maps, core_ids=list(range(N_CORES)),
                               trace=bool(int(os.environ.get("KTRACE", "0"))))
    out = np.concatenate([res.results[i]["out"] for i in range(N_CORES)], axis=0)
    if bool(int(os.environ.get("KTRACE", "0"))):
        kernel.last_exec_time_ns = res.exec_time_ns
        kernel.last_profile = res.profile_json
    return out.astype(np.float32)


# revision 23
# speedup vs baseline: 1.1579x; 1.1579x over previous
"""Trainium2 Bass kernel for nn_AttentionDecoder (Bahdanau attention + GRU greedy decoder).

Sharding: pure data parallel, B=2048 split as 256 rows per core across 8 cores.
All compute in bf16 with f32 PSUM accumulation (verified: rel err ~6e-3, no argmax flips).

Layout scheme (per core, BL=256):
  - partitions packed as p = 32*(b%4) + t  ("bd layout") so block-diagonal attention
    matmuls pack 4 batch rows per matmul; free index g = b//4 makes everything b-major.
  - attention + fc1 feature-major (feature on partitions), GRU gates batch-major,
    bridged by PE transposes.
  - context and energy matmuls run "flipped": enc/att chunks are the stationary
    operand (M=128, dense base-0 PSUM output), attn_bd/v the moving operand.
  - sigmoid computed as (1+tanh(x/2))/2 so tanh/exp/relu/copy share one ACT
    table set (no per-step ACT_TABLE_LOAD).
"""

import os
import threading
import numpy as np
import ml_dtypes

N_CORES = 8
B, T, ENC = 2048, 32, 512
DEC, ATT, EMB, NCLS, L = 256, 256, 64, 37, 10
BL = B // N_CORES  # 256 per core

_BF = ml_dtypes.bfloat16

_lock = threading.Lock()
_cache = {}


def _build():
    import concourse.bass as bass
    import concourse.tile as tile
    from concourse import bacc, mybir

    bf = mybir.dt.bfloat16
    f32 = mybir.dt.float32

    nc = bacc.Bacc("TRN2", target_bir_lowering=False, debug=False,
                   num_devices=N_CORES)

    # ---------------- DRAM parameters ----------------
    d_enc = nc.dram_tensor("enc", [BL, T, ENC], bf, kind="ExternalInput").ap()
    d_wdec = nc.dram_tensor("w_dec", [DEC, ATT], bf, kind="ExternalInput").ap()
    d_wenc = nc.dram_tensor("w_enc", [ENC, ATT], bf, kind="ExternalInput").ap()
    d_v = nc.dram_tensor("v", [ATT, 1], bf, kind="ExternalInput").ap()
    d_embWb = nc.dram_tensor("embWb", [NCLS + 1, 3 * DEC], bf, kind="ExternalInput").ap()
    d_wihc = nc.dram_tensor("w_ih_c", [ENC, 3 * DEC], bf, kind="ExternalInput").ap()
    d_whhrz = nc.dram_tensor("w_hh_rz", [DEC, 2 * DEC], bf, kind="ExternalInput").ap()
    d_whhn = nc.dram_tensor("w_hh_n", [DEC, DEC], bf, kind="ExternalInput").ap()
    d_bhhn = nc.dram_tensor("b_hh_n", [1, DEC], bf, kind="ExternalInput").ap()
    d_fc1h = nc.dram_tensor("fc1_w_h", [DEC, DEC], bf, kind="ExternalInput").ap()
    d_fc1c = nc.dram_tensor("fc1_w_c", [ENC, DEC], bf, kind="ExternalInput").ap()
    d_fc1b = nc.dram_tensor("fc1_b", [DEC, 1], f32, kind="ExternalInput").ap()
    d_fc2w = nc.dram_tensor("fc2_w", [DEC, NCLS], bf, kind="ExternalInput").ap()
    d_fc2b = nc.dram_tensor("fc2_b", [1, NCLS], bf, kind="ExternalInput").ap()
    d_ihw = nc.dram_tensor("init_h_w", [ENC, DEC], bf, kind="ExternalInput").ap()
    d_ihb = nc.dram_tensor("init_h_b", [DEC, 1], f32, kind="ExternalInput").ap()
    d_out = nc.dram_tensor("out", [BL, L, NCLS], f32, kind="ExternalOutput").ap()

    # constants baked into the NEFF
    ident_np = np.eye(128, dtype=_BF)
    d_ident = nc.inline_tensor(ident_np, name="ident").ap()
    onesbd_np = np.zeros((128, 4), dtype=_BF)
    for bs in range(4):
        onesbd_np[32 * bs:32 * bs + 32, bs] = 1.0 / 32.0
    d_onesbd = nc.inline_tensor(onesbd_np, name="onesbd").ap()
    d_onesrow = nc.inline_tensor(np.ones((1, 256), dtype=_BF), name="onesrow").ap()

    AluOp = mybir.AluOpType
    ActF = mybir.ActivationFunctionType

    with tile.TileContext(nc) as tc:
        with (
            tc.tile_pool(name="persist", bufs=1) as P,
            tc.tile_pool(name="wpool", bufs=1) as W,
            tc.tile_pool(name="trans", bufs=3) as TR,
            tc.tile_pool(name="small", bufs=2) as SM,
            tc.tile_pool(name="ps", bufs=3, space="PSUM") as PS,
        ):
            # ---------------- persistent SBUF tensors ----------------
            enc_bd = P.tile([128, 64, ENC], bf, tag="enc_bd")       # 64KB/part
            ep = P.tile([128, 2, T, 256], bf, tag="ep")             # enc_proj^T, t-major
            att = P.tile([128, 2, T, 256], bf, tag="att")           # tanh buffer
            hT = P.tile([128, 2, BL], bf, tag="hT")                 # h feature-major
            h_b = P.tile([128, 2, DEC], bf, tag="h_b")              # h batch-major
            ctxT = P.tile([128, 4, BL], bf, tag="ctxT")             # context feature-major
            onehotT = P.tile([NCLS + 1, BL], bf, tag="onehotT")
            out_sb = P.tile([128, 2, L, NCLS], f32, tag="out_sb")

            # ---------------- weights to SBUF ----------------
            def wload(tag, shape, src, rearr=None):
                t = W.tile(shape, bf, tag=tag)
                nc.sync.dma_start(t[:], src if rearr is None else src.rearrange(rearr, p=128))
                return t

            w_dec = wload("w_dec", [128, 2, ATT], d_wdec, "(k p) n -> p k n")
            w_enc = wload("w_enc", [128, 4, ATT], d_wenc, "(k p) n -> p k n")
            v_sb = wload("v_sb", [128, 2, 1], d_v, "(k p) n -> p k n")
            embWb = wload("embWb", [NCLS + 1, 3 * DEC], d_embWb)
            w_ihc = wload("w_ihc", [128, 4, 3 * DEC], d_wihc, "(k p) n -> p k n")
            w_hhrz = wload("w_hhrz", [128, 2, 2 * DEC], d_whhrz, "(k p) n -> p k n")
            w_hhn = wload("w_hhn", [128, 2, DEC], d_whhn, "(k p) n -> p k n")
            bhhn = wload("bhhn", [1, DEC], d_bhhn)
            fc1h = wload("fc1h", [128, 2, DEC], d_fc1h, "(k p) n -> p k n")
            fc1c = wload("fc1c", [128, 4, DEC], d_fc1c, "(k p) n -> p k n")
            fc2w = wload("fc2w", [128, 2, NCLS], d_fc2w, "(k p) n -> p k n")
            fc2b = wload("fc2b", [1, NCLS], d_fc2b)
            ihw = wload("ihw", [128, 4, DEC], d_ihw, "(k p) n -> p k n")
            ident = wload("ident", [128, 128], d_ident)
            ihb = W.tile([128, 2, 1], f32)
            nc.sync.dma_start(ihb[:], d_ihb.rearrange("(k p) n -> p k n", p=128))
            fc1b = W.tile([128, 2, 1], f32)
            nc.sync.dma_start(fc1b[:], d_fc1b.rearrange("(k p) n -> p k n", p=128))
            ones1 = W.tile([1, 128], bf)
            nc.sync.dma_start(ones1[:], d_onesrow[:, 0:128])

            # ---------------- enc DMA into bd layout ----------------
            # enc_bd[32*bs+t, g, e] = enc[4g+bs, t, e]
            for bs in range(4):
                nc.sync.dma_start(
                    enc_bd[32 * bs:32 * bs + 32, :, :],
                    d_enc[bs::4].rearrange("g t e -> t g e"),
                )

            meanT = TR.tile([128, 4, BL], bf, tag="meanT", bufs=1)
            # ---------------- prologue: enc_proj via xbar DMA transposes ----------------
            # encT (e-major) built by dma_start_transpose per (bt-half, e-block)
            # on the otherwise-idle DMA engines; ep matmuls consume each half.
            d_enc2d = d_enc.rearrange("b t e -> (b t) e")
            for bth in range(2):
                encTh = TR.tile([128, 4, 4096], bf, tag="encTh", bufs=1)
                for eb in range(4):
                    nc.sync.dma_start_transpose(
                        encTh[:, eb, :],
                        d_enc2d[4096 * bth:4096 * bth + 4096,
                                128 * eb:128 * eb + 128])
                # mean over t rides on DVE: encTh free order is bt b-major
                for eb in range(4):
                    mr = TR.tile([128, 4, 128], f32, tag="mr", bufs=1)
                    nc.vector.tensor_reduce(
                        mr[:, eb, :],
                        encTh[:, eb, :].rearrange("p (b t) -> p b t", t=32),
                        axis=mybir.AxisListType.X, op=AluOp.add)
                    nc.vector.tensor_scalar(
                        meanT[:, eb, 128 * bth:128 * bth + 128], mr[:, eb, :],
                        1.0 / 32.0, None, op0=AluOp.mult)
                for c in range(8):
                    for ab in range(2):
                        pp = PS.tile([128, 512], f32, tag="a")
                        for eb in range(4):
                            nc.tensor.matmul(
                                pp[:],
                                w_enc[:, eb, 128 * ab:128 * ab + 128],
                                encTh[:, eb, 512 * c:512 * c + 512],
                                start=(eb == 0), stop=(eb == 3),
                            )
                        # evac to ep (t-major): covers b in [64*bth+16c, +16)
                        b0 = 128 * bth + 16 * c
                        dst = ep[:, ab, :, b0:b0 + 16].rearrange("p t b -> p b t")
                        if ab == 0:
                            nc.vector.tensor_copy(
                                dst, pp[:].rearrange("p (b t) -> p b t", t=32))
                        else:
                            nc.scalar.copy(
                                dst, pp[:].rearrange("p (b t) -> p b t", t=32))

            # ---------------- h0 ----------------
            for db in range(2):
                hp = PS.tile([128, BL], f32, tag="a")
                for eb in range(4):
                    nc.tensor.matmul(hp[:], ihw[:, eb, 128 * db:128 * db + 128],
                                     meanT[:, eb, :], start=(eb == 0), stop=(eb == 3))
                nc.scalar.activation(hT[:, db, :], hp[:], ActF.Tanh, bias=ihb[:, db, :])
            for half in range(2):
                for db in range(2):
                    tp = PS.tile([128, 128], bf, tag="b", bufs=3)
                    nc.tensor.transpose(tp[:], hT[:, db, 128 * half:128 * half + 128],
                                        ident[:])
                    nc.vector.tensor_copy(h_b[:, half, 128 * db:128 * db + 128], tp[:])

            # onehotT init: y0 = 0 -> row 0 ones; row 37 = bias row (always 1)
            nc.vector.memset(onehotT[0:NCLS, :], 0)
            nc.vector.memset(onehotT[0:1, :], 1.0)
            nc.sync.dma_start(onehotT[NCLS:NCLS + 1, :], d_onesrow[:])

            # ---------------- decode loop ----------------
            # emitted per b-half: the two halves form independent dependency
            # chains within a step, so Tile overlaps half-1 elementwise
            # (DVE/ACT) with half-0 matmuls (PE) and vice versa.
            for step in range(L):
                decT = SM.tile([128, 2, BL], bf, tag="decT")
                n_sb = SM.tile([128, 2, DEC], bf, tag="n_sb")
                tz_sb = SM.tile([128, 2, DEC], bf, tag="tz_sb")
                hidT = SM.tile([128, 2, BL], bf, tag="hidT")
                attnT = SM.tile([32, BL], bf, tag="attnT")
                attn_bd = SM.tile([128, 64, 4], bf, tag="attn_bd")
                nc.vector.memset(attn_bd[:], 0)
                for ab in range(2):
                    dp = PS.tile([128, BL], f32, tag="a")
                    for db in range(2):
                        nc.tensor.matmul(dp[:], w_dec[:, db, 128 * ab:128 * ab + 128],
                                         hT[:, db, :], start=(db == 0), stop=(db == 1))
                    nc.scalar.copy(decT[:, ab, :], dp[:])
                for half in range(2):
                    hs, he = 128 * half, 128 * half + 128
                    # s = ep + dec (broadcast over t); tanh in place (one op/half)
                    bcast = decT[:, :, hs:he].rearrange(
                        "p a (o b) -> p a o b", o=1).broadcast_to([128, 2, T, 128])
                    nc.vector.tensor_tensor(att[:, :, :, hs:he], ep[:, :, :, hs:he],
                                            bcast, op=AluOp.add)
                    nc.scalar.activation(att[:, :, :, hs:he], att[:, :, :, hs:he],
                                         ActF.Tanh)
                    # energy (batch-major) via flipped vdot
                    ebp = PS.tile([128, T], f32, tag="b")
                    for t in range(T):
                        for ab in range(2):
                            nc.tensor.matmul(ebp[:, t:t + 1], att[:, ab, t, hs:he],
                                             v_sb[:, ab, :],
                                             start=(ab == 0), stop=(ab == 1))
                    expB = SM.tile([128, T], bf, tag="expB")
                    nc.scalar.activation(expB[:], ebp[:], ActF.Exp)
                    zc = SM.tile([128, 1], f32, tag="zc")
                    nc.vector.tensor_reduce(zc[:], expB[:], axis=mybir.AxisListType.X,
                                            op=AluOp.add)
                    rz = SM.tile([128, 1], f32, tag="rz")
                    nc.vector.reciprocal(rz[:], zc[:])
                    attnB = SM.tile([128, T], bf, tag="attnB")
                    nc.vector.tensor_scalar(attnB[:], expB[:], rz[:], None,
                                            op0=AluOp.mult)
                    tp = PS.tile([32, 128], bf, tag="b")
                    nc.tensor.transpose(tp[:], attnB[:], ident[:])
                    nc.vector.tensor_copy(attnT[:, hs:he], tp[:])
                    # attn blockdiag build for this half
                    for bs in range(4):
                        nc.vector.tensor_copy(
                            attn_bd[32 * bs:32 * bs + 32,
                                    32 * half:32 * half + 32, bs],
                            attnT[:, hs + bs:he:4],
                        )
                    # context feature-major (flipped blockdiag)
                    for eb in range(4):
                        cp = PS.tile([128, 128], f32, tag="cp", bufs=2)
                        for gr in range(32):
                            g = 32 * half + gr
                            nc.tensor.matmul(cp[:, 4 * gr:4 * gr + 4],
                                             enc_bd[:, g, 128 * eb:128 * eb + 128],
                                             attn_bd[:, g, :], start=True, stop=True)
                        nc.vector.tensor_copy(ctxT[:, eb, hs:he], cp[:])
                    # GRU gates
                    gi_rz = PS.tile([128, 2 * DEC], f32, tag="a")
                    gi_n = PS.tile([128, DEC], f32, tag="b")
                    ghn = PS.tile([128, DEC], f32, tag="b")
                    oh = onehotT[:, hs:he]
                    nc.tensor.matmul(gi_rz[:], oh, embWb[:, 0:512],
                                     start=True, stop=False)
                    nc.tensor.matmul(gi_n[:], oh, embWb[:, 512:768],
                                     start=True, stop=False)
                    for eb in range(4):
                        ct = ctxT[:, eb, hs:he]
                        nc.tensor.matmul(gi_rz[:], ct, w_ihc[:, eb, 0:512],
                                         start=False, stop=False)
                        nc.tensor.matmul(gi_n[:], ct, w_ihc[:, eb, 512:768],
                                         start=False, stop=False)
                    nc.tensor.matmul(ghn[:], ones1[:], bhhn[:], start=True, stop=False)
                    nc.tensor.matmul(gi_n[:], ones1[:], bhhn[:], start=False, stop=False)
                    for db in range(2):
                        hTs = hT[:, db, hs:he]
                        nc.tensor.matmul(gi_rz[:], hTs, w_hhrz[:, db, :], start=False,
                                         stop=(db == 1))
                        nc.tensor.matmul(ghn[:], hTs, w_hhn[:, db, :], start=False,
                                         stop=(db == 1))
                        nc.tensor.matmul(gi_n[:], hTs, w_hhn[:, db, :], start=False,
                                         stop=(db == 1))
                    # r-gate via tanh: npre = gi_n + tanh(rx/2)*ghn2
                    tr_sb = SM.tile([128, DEC], bf, tag="tr_sb")
                    nc.scalar.activation(tr_sb[:], gi_rz[:, 0:DEC], ActF.Tanh, scale=0.5)
                    nc.scalar.activation(tz_sb[:, half, :], gi_rz[:, DEC:2 * DEC],
                                         ActF.Tanh, scale=0.5)
                    rhn = SM.tile([128, DEC], bf, tag="rhn")
                    nc.vector.tensor_tensor(rhn[:], tr_sb[:], ghn[:], op=AluOp.mult)
                    npre = SM.tile([128, DEC], bf, tag="npre")
                    nc.vector.tensor_tensor(npre[:], gi_n[:], rhn[:], op=AluOp.add)
                    nc.scalar.activation(n_sb[:, half, :], npre[:], ActF.Tanh)
                    # h_new = 0.5*(t1 + tz*t1) + n,  t1 = h - n
                    t1 = SM.tile([128, DEC], bf, tag="t1")
                    nc.vector.tensor_tensor(t1[:], h_b[:, half, :], n_sb[:, half, :],
                                            op=AluOp.subtract)
                    t2 = SM.tile([128, DEC], bf, tag="t2")
                    nc.vector.tensor_tensor(t2[:], tz_sb[:, half, :], t1[:],
                                            op=AluOp.mult)
                    t3 = SM.tile([128, DEC], bf, tag="t3")
                    nc.vector.tensor_tensor(t3[:], t1[:], t2[:], op=AluOp.add)
                    nc.vector.scalar_tensor_tensor(h_b[:, half, :], t3[:], 0.5,
                                                   n_sb[:, half, :],
                                                   op0=AluOp.mult, op1=AluOp.add)
                    for db in range(2):
                        tp = PS.tile([128, 128], bf, tag="b")
                        nc.tensor.transpose(tp[:],
                                            h_b[:, half, 128 * db:128 * db + 128],
                                            ident[:])
                        nc.vector.tensor_copy(hT[:, db, hs:he], tp[:])
                    # fc1 feature-major, full-b (only once, after both halves)
                    if half == 1:
                        for db in range(2):
                            fp = PS.tile([128, BL], f32, tag="a")
                            for k in range(2):
                                nc.tensor.matmul(fp[:],
                                                 fc1h[:, k, 128 * db:128 * db + 128],
                                                 hT[:, k, :], start=(k == 0), stop=False)
                            for eb in range(4):
                                nc.tensor.matmul(fp[:],
                                                 fc1c[:, eb, 128 * db:128 * db + 128],
                                                 ctxT[:, eb, :], start=False,
                                                 stop=(eb == 3))
                            nc.scalar.activation(hidT[:, db, :], fp[:], ActF.Relu,
                                                 bias=fc1b[:, db, :])
                    if half == 1:
                        for h2 in range(2):
                            h2s, h2e = 128 * h2, 128 * h2 + 128
                            lp = PS.tile([128, NCLS], f32, tag="b")
                            nc.tensor.matmul(lp[:], ones1[:], fc2b[:],
                                             start=True, stop=False)
                            for db in range(2):
                                nc.tensor.matmul(lp[:], hidT[:, db, h2s:h2e],
                                                 fc2w[:, db, :], start=False,
                                                 stop=(db == 1))
                            nc.scalar.copy(out_sb[:, h2, step, :], lp[:])
                            if step < L - 1:
                                mx = SM.tile([128, 1], f32, tag="zc")
                                nc.vector.tensor_reduce(mx[:], lp[:],
                                                        axis=mybir.AxisListType.X,
                                                        op=AluOp.max)
                                ohB = SM.tile([128, NCLS], bf, tag="ohB")
                                nc.vector.tensor_tensor(
                                    ohB[:], lp[:],
                                    mx[:].broadcast_to([128, NCLS]), op=AluOp.is_equal)
                                tp = PS.tile([NCLS, 128], bf, tag="b")
                                nc.tensor.transpose(tp[:], ohB[:], ident[:])
                                nc.vector.tensor_copy(onehotT[0:NCLS, h2s:h2e], tp[:])

            # ---------------- output DMA ----------------
            for half in range(2):
                nc.sync.dma_start(
                    d_out[128 * half:128 * half + 128],
                    out_sb[:, half, :, :],
                )

    nc.compile()
    return nc


def _get_nc():
    with _lock:
        if "nc" not in _cache:
            _cache["nc"] = _build()
        return _cache["nc"]


def kernel(**inputs):
    nc = _get_nc()
    from concourse.bass_utils import run_bass_kernel_spmd

    enc = np.ascontiguousarray(inputs["encoder_outputs"], dtype=np.float32)
    emb = inputs["emb"].astype(np.float32)
    W_enc = inputs["W_enc"].astype(np.float32)
    W_dec = inputs["W_dec"].astype(np.float32)
    v = inputs["v"].astype(np.float32)
    init_h_W = inputs["init_h_W"].astype(np.float32)
    init_h_b = inputs["init_h_b"].astype(np.float32)
    W_ih = inputs["W_ih"].astype(np.float32)
    b_ih = inputs["b_ih"].astype(np.float32)
    W_hh = inputs["W_hh"].astype(np.float32)
    b_hh = inputs["b_hh"].astype(np.float32)
    fc1_W = inputs["fc1_W"].astype(np.float32)
    fc1_b = inputs["fc1_b"].astype(np.float32)
    fc2_W = inputs["fc2_W"].astype(np.float32)
    fc2_b = inputs["fc2_b"].astype(np.float32)

    # host precompute: embedding projected through W_ih (emb part) + rz biases;
    # W_hh_n/b_hh_n halved for the tanh-form sigmoid r-gate
    bias_row = np.concatenate([(b_ih + b_hh)[:2 * DEC], b_ih[2 * DEC:]])
    embWb = np.concatenate([emb @ W_ih[:EMB], bias_row[None, :]], axis=0)

    bfc = lambda a: np.ascontiguousarray(a, dtype=_BF)
    shared = {
        "w_dec": bfc(W_dec),
        "w_enc": bfc(W_enc),
        "v": bfc(v.reshape(ATT, 1)),
        "embWb": bfc(embWb),
        "w_ih_c": bfc(W_ih[EMB:]),
        "w_hh_rz": bfc(W_hh[:, :2 * DEC]),
        "w_hh_n": bfc(0.5 * W_hh[:, 2 * DEC:]),
        "b_hh_n": bfc(0.5 * b_hh[2 * DEC:].reshape(1, DEC)),
        "fc1_w_h": bfc(fc1_W[:DEC]),
        "fc1_w_c": bfc(fc1_W[DEC:]),
        "fc1_b": np.ascontiguousarray(fc1_b.reshape(DEC, 1), dtype=np.float32),
        "fc2_w": bfc(fc2_W),
        "fc2_b": bfc(fc2_b.reshape(1, NCLS)),
        "init_h_w": bfc(init_h_W),
        "init_h_b": np.ascontiguousarray(init_h_b.reshape(DEC, 1), dtype=np.float32),
    }
    enc_bf = enc.astype(_BF)
    in_maps = []
    for i in range(N_CORES):
        m = dict(shared)
        m["enc"] = np.ascontiguousarray(enc_bf[i * BL:(i + 1) * BL])
        in_maps.append(m)

    res = run_bass_kernel_spmd(nc, in_maps, core_ids=list(range(N_CORES)),
                               trace=bool(int(os.environ.get("KTRACE", "0"))))
    out = np.concatenate([res.results[i]["out"] for i in range(N_CORES)], axis=0)
    if bool(int(os.environ.get("KTRACE", "0"))):
        kernel.last_exec_time_ns = res.exec_time_ns
        kernel.last_profile = res.profile_json
    return out.astype(np.float32)


# revision 25
# speedup vs baseline: 1.2591x; 1.0874x over previous
"""Trainium2 Bass kernel for nn_AttentionDecoder (Bahdanau attention + GRU greedy decoder).

Sharding: pure data parallel, B=2048 split as 256 rows per core across 8 cores.
All compute in bf16 with f32 PSUM accumulation (verified: rel err ~6e-3, no argmax flips).

Layout scheme (per core, BL=256):
  - partitions packed as p = 32*(b%4) + t  ("bd layout") so block-diagonal attention
    matmuls pack 4 batch rows per matmul; free index g = b//4 makes everything b-major.
  - attention + fc1 feature-major (feature on partitions), GRU gates batch-major,
    bridged by PE transposes.
  - context and energy matmuls run "flipped": enc/att chunks are the stationary
    operand (M=128, dense base-0 PSUM output), attn_bd/v the moving operand.
  - sigmoid computed as (1+tanh(x/2))/2 so tanh/exp/relu/copy share one ACT
    table set (no per-step ACT_TABLE_LOAD).
"""

import os
import threading
import numpy as np
import ml_dtypes

N_CORES = 8
B, T, ENC = 2048, 32, 512
DEC, ATT, EMB, NCLS, L = 256, 256, 64, 37, 10
BL = B // N_CORES  # 256 per core

_BF = ml_dtypes.bfloat16

_lock = threading.Lock()
_cache = {}


def _build():
    import concourse.bass as bass
    import concourse.tile as tile
    from concourse import bacc, mybir

    bf = mybir.dt.bfloat16
    f32 = mybir.dt.float32

    nc = bacc.Bacc("TRN2", target_bir_lowering=False, debug=False,
                   num_devices=N_CORES)

    # ---------------- DRAM parameters ----------------
    d_enc = nc.dram_tensor("enc", [BL, T, ENC], bf, kind="ExternalInput").ap()
    d_wdec = nc.dram_tensor("w_dec", [DEC, ATT], bf, kind="ExternalInput").ap()
    d_wenc = nc.dram_tensor("w_enc", [ENC, ATT], bf, kind="ExternalInput").ap()
    d_v = nc.dram_tensor("v", [ATT, 1], bf, kind="ExternalInput").ap()
    d_embWb = nc.dram_tensor("embWb", [NCLS + 1, 3 * DEC], bf, kind="ExternalInput").ap()
    d_wihc = nc.dram_tensor("w_ih_c", [ENC, 3 * DEC], bf, kind="ExternalInput").ap()
    d_whhrz = nc.dram_tensor("w_hh_rz", [DEC, 2 * DEC], bf, kind="ExternalInput").ap()
    d_whhn = nc.dram_tensor("w_hh_n", [DEC, DEC], bf, kind="ExternalInput").ap()
    d_bhhn = nc.dram_tensor("b_hh_n", [1, DEC], bf, kind="ExternalInput").ap()
    d_fc1h = nc.dram_tensor("fc1_w_h", [DEC, DEC], bf, kind="ExternalInput").ap()
    d_fc1c = nc.dram_tensor("fc1_w_c", [ENC, DEC], bf, kind="ExternalInput").ap()
    d_fc1b = nc.dram_tensor("fc1_b", [DEC, 1], f32, kind="ExternalInput").ap()
    d_fc2w = nc.dram_tensor("fc2_w", [DEC, NCLS], bf, kind="ExternalInput").ap()
    d_fc2b = nc.dram_tensor("fc2_b", [1, NCLS], bf, kind="ExternalInput").ap()
    d_ihw = nc.dram_tensor("init_h_w", [ENC, DEC], bf, kind="ExternalInput").ap()
    d_ihb = nc.dram_tensor("init_h_b", [DEC, 1], f32, kind="ExternalInput").ap()
    d_out = nc.dram_tensor("out", [BL, L, NCLS], f32, kind="ExternalOutput").ap()

    # constants baked into the NEFF
    ident_np = np.eye(128, dtype=_BF)
    d_ident = nc.inline_tensor(ident_np, name="ident").ap()
    onesbd_np = np.zeros((128, 4), dtype=_BF)
    for bs in range(4):
        onesbd_np[32 * bs:32 * bs + 32, bs] = 1.0 / 32.0
    d_onesbd = nc.inline_tensor(onesbd_np, name="onesbd").ap()
    d_onesrow = nc.inline_tensor(np.ones((1, 256), dtype=_BF), name="onesrow").ap()

    AluOp = mybir.AluOpType
    ActF = mybir.ActivationFunctionType

    with tile.TileContext(nc) as tc:
        with (
            tc.tile_pool(name="persist", bufs=1) as P,
            tc.tile_pool(name="wpool", bufs=1) as W,
            tc.tile_pool(name="trans", bufs=3) as TR,
            tc.tile_pool(name="small", bufs=2) as SM,
            tc.tile_pool(name="ps", bufs=3, space="PSUM") as PS,
        ):
            # ---------------- persistent SBUF tensors ----------------
            enc_bd = P.tile([128, 64, ENC], bf, tag="enc_bd")       # 64KB/part
            ep = P.tile([128, 2, T, 256], bf, tag="ep")             # enc_proj^T, t-major
            att = P.tile([128, 2, T, 256], bf, tag="att")           # tanh buffer
            hT = P.tile([128, 2, BL], bf, tag="hT")                 # h feature-major
            h_b = P.tile([128, 2, DEC], bf, tag="h_b")              # h batch-major
            ctxT = P.tile([128, 4, BL], bf, tag="ctxT")             # context feature-major
            onehotT = P.tile([NCLS + 1, BL], bf, tag="onehotT")
            out_sb = P.tile([128, 2, L, NCLS], f32, tag="out_sb")

            # ---------------- weights to SBUF ----------------
            def wload(tag, shape, src, rearr=None):
                t = W.tile(shape, bf, tag=tag)
                nc.sync.dma_start(t[:], src if rearr is None else src.rearrange(rearr, p=128))
                return t

            w_dec = wload("w_dec", [128, 2, ATT], d_wdec, "(k p) n -> p k n")
            w_enc = wload("w_enc", [128, 4, ATT], d_wenc, "(k p) n -> p k n")
            v_sb = wload("v_sb", [128, 2, 1], d_v, "(k p) n -> p k n")
            embWb = wload("embWb", [NCLS + 1, 3 * DEC], d_embWb)
            w_ihc = wload("w_ihc", [128, 4, 3 * DEC], d_wihc, "(k p) n -> p k n")
            w_hhrz = wload("w_hhrz", [128, 2, 2 * DEC], d_whhrz, "(k p) n -> p k n")
            w_hhn = wload("w_hhn", [128, 2, DEC], d_whhn, "(k p) n -> p k n")
            bhhn = wload("bhhn", [1, DEC], d_bhhn)
            fc1h = wload("fc1h", [128, 2, DEC], d_fc1h, "(k p) n -> p k n")
            fc1c = wload("fc1c", [128, 4, DEC], d_fc1c, "(k p) n -> p k n")
            fc2w = wload("fc2w", [128, 2, NCLS], d_fc2w, "(k p) n -> p k n")
            fc2b = wload("fc2b", [1, NCLS], d_fc2b)
            ihw = wload("ihw", [128, 4, DEC], d_ihw, "(k p) n -> p k n")
            ident = wload("ident", [128, 128], d_ident)
            ihb = W.tile([128, 2, 1], f32)
            nc.sync.dma_start(ihb[:], d_ihb.rearrange("(k p) n -> p k n", p=128))
            fc1b = W.tile([128, 2, 1], f32)
            nc.sync.dma_start(fc1b[:], d_fc1b.rearrange("(k p) n -> p k n", p=128))
            ones1 = W.tile([1, 128], bf)
            nc.sync.dma_start(ones1[:], d_onesrow[:, 0:128])

            # ---------------- enc DMA into bd layout ----------------
            # enc_bd[32*bs+t, g, e] = enc[4g+bs, t, e]
            for bs in range(4):
                nc.sync.dma_start(
                    enc_bd[32 * bs:32 * bs + 32, :, :],
                    d_enc[bs::4].rearrange("g t e -> t g e"),
                )

            meanT = TR.tile([128, 4, BL], bf, tag="meanT", bufs=1)
            # ---------------- prologue: enc_proj via xbar DMA transposes ----------------
            # encT (e-major) built by dma_start_transpose per (bt-half, e-block)
            # on the otherwise-idle DMA engines; ep matmuls consume each half.
            d_enc2d = d_enc.rearrange("b t e -> (b t) e")
            for bth in range(2):
                encTh = TR.tile([128, 4, 4096], bf, tag="encTh", bufs=1)
                for eb in range(4):
                    nc.sync.dma_start_transpose(
                        encTh[:, eb, :],
                        d_enc2d[4096 * bth:4096 * bth + 4096,
                                128 * eb:128 * eb + 128])
                # mean over t rides on DVE: encTh free order is bt b-major
                for eb in range(4):
                    mr = TR.tile([128, 4, 128], f32, tag="mr", bufs=1)
                    nc.vector.tensor_reduce(
                        mr[:, eb, :],
                        encTh[:, eb, :].rearrange("p (b t) -> p b t", t=32),
                        axis=mybir.AxisListType.X, op=AluOp.add)
                    nc.vector.tensor_scalar(
                        meanT[:, eb, 128 * bth:128 * bth + 128], mr[:, eb, :],
                        1.0 / 32.0, None, op0=AluOp.mult)
                for c in range(8):
                    for ab in range(2):
                        pp = PS.tile([128, 512], f32, tag="a")
                        for eb in range(4):
                            nc.tensor.matmul(
                                pp[:],
                                w_enc[:, eb, 128 * ab:128 * ab + 128],
                                encTh[:, eb, 512 * c:512 * c + 512],
                                start=(eb == 0), stop=(eb == 3),
                            )
                        # evac to ep (t-major): covers b in [64*bth+16c, +16)
                        b0 = 128 * bth + 16 * c
                        dst = ep[:, ab, :, b0:b0 + 16].rearrange("p t b -> p b t")
                        if ab == 0:
                            nc.vector.tensor_copy(
                                dst, pp[:].rearrange("p (b t) -> p b t", t=32))
                        else:
                            nc.scalar.copy(
                                dst, pp[:].rearrange("p (b t) -> p b t", t=32))

            # ---------------- h0 ----------------
            for db in range(2):
                hp = PS.tile([128, BL], f32, tag="a")
                for eb in range(4):
                    nc.tensor.matmul(hp[:], ihw[:, eb, 128 * db:128 * db + 128],
                                     meanT[:, eb, :], start=(eb == 0), stop=(eb == 3))
                nc.scalar.activation(hT[:, db, :], hp[:], ActF.Tanh, bias=ihb[:, db, :])
            for half in range(2):
                for db in range(2):
                    tp = PS.tile([128, 128], bf, tag="b", bufs=3)
                    nc.tensor.transpose(tp[:], hT[:, db, 128 * half:128 * half + 128],
                                        ident[:])
                    nc.vector.tensor_copy(h_b[:, half, 128 * db:128 * db + 128], tp[:])

            # onehotT init: y0 = 0 -> row 0 ones; row 37 = bias row (always 1)
            nc.vector.memset(onehotT[0:NCLS, :], 0)
            nc.vector.memset(onehotT[0:1, :], 1.0)
            nc.sync.dma_start(onehotT[NCLS:NCLS + 1, :], d_onesrow[:])

            # ---------------- decode loop ----------------
            # emitted per b-half: the two halves form independent dependency
            # chains within a step, so Tile overlaps half-1 elementwise
            # (DVE/ACT) with half-0 matmuls (PE) and vice versa.
            for step in range(L):
                decT = SM.tile([128, 2, BL], bf, tag="decT")
                n_sb = SM.tile([128, 2, DEC], bf, tag="n_sb")
                tz_sb = SM.tile([128, 2, DEC], bf, tag="tz_sb")
                hidT = SM.tile([128, 2, BL], bf, tag="hidT")
                attnT = SM.tile([32, BL], bf, tag="attnT")
                attn_bd = SM.tile([128, 64, 4], bf, tag="attn_bd")
                nc.vector.memset(attn_bd[:], 0)
                for ab in range(2):
                    dp = PS.tile([128, BL], f32, tag="a")
                    for db in range(2):
                        nc.tensor.matmul(dp[:], w_dec[:, db, 128 * ab:128 * ab + 128],
                                         hT[:, db, :], start=(db == 0), stop=(db == 1))
                    nc.scalar.copy(decT[:, ab, :], dp[:])
                for half in range(2):
                    hs, he = 128 * half, 128 * half + 128
                    # s = ep + dec (broadcast over t); tanh in place
                    for ab in range(2):
                        bcast = decT[:, ab, hs:he].rearrange(
                            "p (o b) -> p o b", o=1).broadcast_to([128, T, 128])
                        nc.vector.tensor_tensor(att[:, ab, :, hs:he],
                                                ep[:, ab, :, hs:he], bcast,
                                                op=AluOp.add)
                        nc.scalar.activation(att[:, ab, :, hs:he],
                                             att[:, ab, :, hs:he], ActF.Tanh)
                    # energy (batch-major) via flipped vdot
                    ebp = PS.tile([128, T], f32, tag="b")
                    for t in range(T):
                        for ab in range(2):
                            nc.tensor.matmul(ebp[:, t:t + 1], att[:, ab, t, hs:he],
                                             v_sb[:, ab, :],
                                             start=(ab == 0), stop=(ab == 1))
                    expB = SM.tile([128, T], bf, tag="expB")
                    nc.scalar.activation(expB[:], ebp[:], ActF.Exp)
                    zc = SM.tile([128, 1], f32, tag="zc")
                    nc.vector.tensor_reduce(zc[:], expB[:], axis=mybir.AxisListType.X,
                                            op=AluOp.add)
                    rz = SM.tile([128, 1], f32, tag="rz")
                    nc.vector.reciprocal(rz[:], zc[:])
                    attnB = SM.tile([128, T], bf, tag="attnB")
                    nc.vector.tensor_scalar(attnB[:], expB[:], rz[:], None,
                                            op0=AluOp.mult)
                    tp = PS.tile([32, 128], bf, tag="b")
                    nc.tensor.transpose(tp[:], attnB[:], ident[:])
                    nc.vector.tensor_copy(attnT[:, hs:he], tp[:])
                    # attn blockdiag build for this half
                    for bs in range(4):
                        nc.vector.tensor_copy(
                            attn_bd[32 * bs:32 * bs + 32,
                                    32 * half:32 * half + 32, bs],
                            attnT[:, hs + bs:he:4],
                        )
                    # context feature-major (flipped blockdiag)
                    for eb in range(4):
                        cp = PS.tile([128, 128], f32, tag="cp", bufs=2)
                        for gr in range(32):
                            g = 32 * half + gr
                            nc.tensor.matmul(cp[:, 4 * gr:4 * gr + 4],
                                             enc_bd[:, g, 128 * eb:128 * eb + 128],
                                             attn_bd[:, g, :], start=True, stop=True)
                        nc.vector.tensor_copy(ctxT[:, eb, hs:he], cp[:])
                    # GRU gates
                    gi_rz = PS.tile([128, 2 * DEC], f32, tag="a")
                    gi_n = PS.tile([128, DEC], f32, tag="b")
                    ghn = PS.tile([128, DEC], f32, tag="b")
                    oh = onehotT[:, hs:he]
                    nc.tensor.matmul(gi_rz[:], oh, embWb[:, 0:512],
                                     start=True, stop=False)
                    nc.tensor.matmul(gi_n[:], oh, embWb[:, 512:768],
                                     start=True, stop=False)
                    for eb in range(4):
                        ct = ctxT[:, eb, hs:he]
                        nc.tensor.matmul(gi_rz[:], ct, w_ihc[:, eb, 0:512],
                                         start=False, stop=False)
                        nc.tensor.matmul(gi_n[:], ct, w_ihc[:, eb, 512:768],
                                         start=False, stop=False)
                    nc.tensor.matmul(ghn[:], ones1[:], bhhn[:], start=True, stop=False)
                    nc.tensor.matmul(gi_n[:], ones1[:], bhhn[:], start=False, stop=False)
                    for db in range(2):
                        hTs = hT[:, db, hs:he]
                        nc.tensor.matmul(gi_rz[:], hTs, w_hhrz[:, db, :], start=False,
                                         stop=(db == 1))
                        nc.tensor.matmul(ghn[:], hTs, w_hhn[:, db, :], start=False,
                                         stop=(db == 1))
                        nc.tensor.matmul(gi_n[:], hTs, w_hhn[:, db, :], start=False,
                                         stop=(db == 1))
                    # r-gate via tanh: npre = gi_n + tanh(rx/2)*ghn2
                    tr_sb = SM.tile([128, DEC], bf, tag="tr_sb")
                    nc.scalar.activation(tr_sb[:], gi_rz[:, 0:DEC], ActF.Tanh, scale=0.5)
                    nc.scalar.activation(tz_sb[:, half, :], gi_rz[:, DEC:2 * DEC],
                                         ActF.Tanh, scale=0.5)
                    rhn = SM.tile([128, DEC], bf, tag="rhn")
                    nc.vector.tensor_tensor(rhn[:], tr_sb[:], ghn[:], op=AluOp.mult)
                    npre = SM.tile([128, DEC], bf, tag="npre")
                    nc.vector.tensor_tensor(npre[:], gi_n[:], rhn[:], op=AluOp.add)
                    nc.scalar.activation(n_sb[:, half, :], npre[:], ActF.Tanh)
                    # h_new = 0.5*(t1 + tz*t1) + n,  t1 = h - n
                    t1 = SM.tile([128, DEC], bf, tag="t1")
                    nc.vector.tensor_tensor(t1[:], h_b[:, half, :], n_sb[:, half, :],
                                            op=AluOp.subtract)
                    t2 = SM.tile([128, DEC], bf, tag="t2")
                    nc.vector.tensor_tensor(t2[:], tz_sb[:, half, :], t1[:],
                                            op=AluOp.mult)
                    t3 = SM.tile([128, DEC], bf, tag="t3")
                    nc.vector.tensor_tensor(t3[:], t1[:], t2[:], op=AluOp.add)
                    nc.vector.scalar_tensor_tensor(h_b[:, half, :], t3[:], 0.5,
                                                   n_sb[:, half, :],
                                                   op0=AluOp.mult, op1=AluOp.add)
                    for db in range(2):
                        tp = PS.tile([128, 128], bf, tag="b")
                        nc.tensor.transpose(tp[:],
                                            h_b[:, half, 128 * db:128 * db + 128],
                                            ident[:])
                        nc.vector.tensor_copy(hT[:, db, hs:he], tp[:])
                    # fc1 feature-major, full-b (only once, after both halves)
                    if half == 1:
                        for db in range(2):
                            fp = PS.tile([128, BL], f32, tag="a")
                            for k in range(2):
                                nc.tensor.matmul(fp[:],
                                                 fc1h[:, k, 128 * db:128 * db + 128],
                                                 hT[:, k, :], start=(k == 0), stop=False)
                            for eb in range(4):
                                nc.tensor.matmul(fp[:],
                                                 fc1c[:, eb, 128 * db:128 * db + 128],
                                                 ctxT[:, eb, :], start=False,
                                                 stop=(eb == 3))
                            nc.scalar.activation(hidT[:, db, :], fp[:], ActF.Relu,
                                                 bias=fc1b[:, db, :])
                    if half == 1:
                        for h2 in range(2):
                            h2s, h2e = 128 * h2, 128 * h2 + 128
                            lp = PS.tile([128, NCLS], f32, tag="b")
                            nc.tensor.matmul(lp[:], ones1[:], fc2b[:],
                                             start=True, stop=False)
                            for db in range(2):
                                nc.tensor.matmul(lp[:], hidT[:, db, h2s:h2e],
                                                 fc2w[:, db, :], start=False,
                                                 stop=(db == 1))
                            nc.scalar.copy(out_sb[:, h2, step, :], lp[:])
                            if step < L - 1:
                                mx = SM.tile([128, 1], f32, tag="zc")
                                nc.vector.tensor_reduce(mx[:], lp[:],
                                                        axis=mybir.AxisListType.X,
                                                        op=AluOp.max)
                                ohB = SM.tile([128, NCLS], bf, tag="ohB")
                                nc.vector.tensor_tensor(
                                    ohB[:], lp[:],
                                    mx[:].broadcast_to([128, NCLS]), op=AluOp.is_equal)
                                tp = PS.tile([NCLS, 128], bf, tag="b")
                                nc.tensor.transpose(tp[:], ohB[:], ident[:])
                                nc.vector.tensor_copy(onehotT[0:NCLS, h2s:h2e], tp[:])

            # ---------------- output DMA ----------------
            for half in range(2):
                nc.sync.dma_start(
                    d_out[128 * half:128 * half + 128],
                    out_sb[:, half, :, :],
                )

    nc.compile()
    return nc


def _get_nc():
    with _lock:
        if "nc" not in _cache:
            _cache["nc"] = _build()
        return _cache["nc"]


def kernel(**inputs):
    nc = _get_nc()
    from concourse.bass_utils import run_bass_kernel_spmd

    enc = np.ascontiguousarray(inputs["encoder_outputs"], dtype=np.float32)
    emb = inputs["emb"].astype(np.float32)
    W_enc = inputs["W_enc"].astype(np.float32)
    W_dec = inputs["W_dec"].astype(np.float32)
    v = inputs["v"].astype(np.float32)
    init_h_W = inputs["init_h_W"].astype(np.float32)
    init_h_b = inputs["init_h_b"].astype(np.float32)
    W_ih = inputs["W_ih"].astype(np.float32)
    b_ih = inputs["b_ih"].astype(np.float32)
    W_hh = inputs["W_hh"].astype(np.float32)
    b_hh = inputs["b_hh"].astype(np.float32)
    fc1_W = inputs["fc1_W"].astype(np.float32)
    fc1_b = inputs["fc1_b"].astype(np.float32)
    fc2_W = inputs["fc2_W"].astype(np.float32)
    fc2_b = inputs["fc2_b"].astype(np.float32)

    # host precompute: embedding projected through W_ih (emb part) + rz biases;
    # W_hh_n/b_hh_n halved for the tanh-form sigmoid r-gate
    bias_row = np.concatenate([(b_ih + b_hh)[:2 * DEC], b_ih[2 * DEC:]])
    embWb = np.concatenate([emb @ W_ih[:EMB], bias_row[None, :]], axis=0)

    bfc = lambda a: np.ascontiguousarray(a, dtype=_BF)
    shared = {
        "w_dec": bfc(W_dec),
        "w_enc": bfc(W_enc),
        "v": bfc(v.reshape(ATT, 1)),
        "embWb": bfc(embWb),
        "w_ih_c": bfc(W_ih[EMB:]),
        "w_hh_rz": bfc(W_hh[:, :2 * DEC]),
        "w_hh_n": bfc(0.5 * W_hh[:, 2 * DEC:]),
        "b_hh_n": bfc(0.5 * b_hh[2 * DEC:].reshape(1, DEC)),
        "fc1_w_h": bfc(fc1_W[:DEC]),
        "fc1_w_c": bfc(fc1_W[DEC:]),
        "fc1_b": np.ascontiguousarray(fc1_b.reshape(DEC, 1), dtype=np.float32),
        "fc2_w": bfc(fc2_W),
        "fc2_b": bfc(fc2_b.reshape(1, NCLS)),
        "init_h_w": bfc(init_h_W),
        "init_h_b": np.ascontiguousarray(init_h_b.reshape(DEC, 1), dtype=np.float32),
    }
    enc_bf = enc.astype(_BF)
    in_maps = []
    for i in range(N_CORES):
        m = dict(shared)
        m["enc"] = np.ascontiguousarray(enc_bf[i * BL:(i + 1) * BL])
        in_maps.append(m)

    res = run_bass_kernel_spmd(nc, in_maps, core_ids=list(range(N_CORES)),
                               trace=bool(int(os.environ.get("KTRACE", "0"))))
    out = np.concatenate([res.results[i]["out"] for i in range(N_CORES)], axis=0)
    if bool(int(os.environ.get("KTRACE", "0"))):
        kernel.last_exec_time_ns = res.exec_time_ns
        kernel.last_profile = res.profile_json
    return out.astype(np.float32)
